# revision 63
# baseline (speedup 1.0000x reference)
"""ConformerBlock Trainium2 kernel.

Data-parallel over batch: B=8 = one batch element per NeuronCore, no
collectives (every module in the block is per-sample, including the
GroupNorm which normalizes over (C,T) of each sample).

Per-core layout strategy:
  - residual `y` kept time-major [T=1024, D=512] as [128, 8, 512] SBUF tile
  - LayerNorm stats via bn_stats per 128-row time tile; gamma/beta folded
    into the *following* matmul's weights on the host. The LN + transpose
    for the NEXT module is emitted inline (lagged 2 tiles) inside each
    module's residual-update loop so the in-order PE queue never drains at
    module boundaries (keeps the HAM clock gate at 8/8).
  - normalized tiles are PE-transposed to feature-major [D, T]; weights
    stationary bf16, activations moving bf16, N=512 per matmul. Output
    biases are applied as K=1 rank-1 (ones-row) matmuls into the same
    PSUM accumulation, keeping bias adds off the critical DVE/Pool path.
  - windowed attention (|i-j|<=32) computed k-major: scores [k, q] via
    3 kpad-tile matmuls per 256-query block (sb pair batched in one PSUM
    bank), exp on ScalarE, multiplicative 0/1 masks on DVE (bf16 2x),
    denominators via an M=64 ones matmul (broadcast across partitions),
    reciprocal_approx_fast, unnormalized AV, then one [64,256] multiply.
    No probability transposes and no PSUM->SBUF probability copies.
  - depthwise conv K=31: 27 taps as accumulating diagonal matmuls on the
    PE (weights streamed from DRAM), 4 even-offset taps as two parallel
    bf16 multiply-add chains on the DVE, joined at the PSUM drain.
  - weight prefetches ride the scalar-engine DMA queue so they overlap
    the x/residual load on the sync queue.
"""

import numpy as np
import ml_dtypes

import concourse.bass as bass
import concourse.bacc as bacc
import concourse.tile as tile
from concourse import mybir
from concourse.bass_utils import run_bass_kernel_spmd
from concourse import bass_isa

F32 = mybir.dt.float32
F32R = mybir.dt.float32r
BF16 = mybir.dt.bfloat16
NP_BF16 = ml_dtypes.bfloat16
AF = mybir.ActivationFunctionType
OP = mybir.AluOpType
AX = mybir.AxisListType

B, T, D, H, KTAP, WIN = 8, 1024, 512, 8, 31, 64
DF = 4 * D            # 2048 ffn hidden
DC = 2 * D            # 1024 conv channels
DH = D // H           # 64
EPS = 1e-5
P = 128
TT_N = T // P         # 8 time tiles
D_T = D // P          # 4
DF_T = DF // P        # 16
DC_T = DC // P        # 8
PAD = 32              # ln_t / kpad leading pad
KP_W = PAD + T + 96   # 1152 padded time width (feature-major)
CPAD = 15             # conv halo
G_W = T + 2 * CPAD    # 1054
NEG = -30000.0

N_CORES = 8

# depthwise taps: a few even offsets run on the DVE (two parallel
# accumulation chains), the rest accumulate on the PE as diagonal matmuls
DVE_TAPS = [4, 8, 12, 16]
PE_TAPS = [j for j in range(KTAP) if j not in DVE_TAPS]
N_PE_TAP = len(PE_TAPS)
N_DVE_TAP = len(DVE_TAPS)


def _dram_vec_bcast_ap(dram_ap, n):
    """AP reading a [n] dram vector broadcast across 128 partitions."""
    return bass.AP(tensor=dram_ap.tensor, offset=dram_ap.offset,
                   ap=[[0, P], [1, n]])


def _build(nc):
    dp = nc.declare_dram_parameter
    x_d = dp("x", [T, D], F32, isOutput=False)
    w1f_d = dp("w1f", [D, DF], BF16, isOutput=False)
    b1f_d = dp("b1f", [P, DF_T], F32, isOutput=False)
    w2_d = dp("w2", [DF, D], BF16, isOutput=False)
    b2r_d = dp("b2r", [1, D], BF16, isOutput=False)
    qkvw_d = dp("qkvw", [D, 3 * D], BF16, isOutput=False)
    qb_d = dp("qb", [P, 4], F32, isOutput=False)
    kb_d = dp("kb", [P, 4], F32, isOutput=False)
    outw_d = dp("outw", [D, D], BF16, isOutput=False)
    outbr_d = dp("outbr", [1, D], BF16, isOutput=False)
    pw1t_d = dp("pw1t", [D, 2 * DC], BF16, isOutput=False)
    ba1_d = dp("ba1", [P, DC_T], F32, isOutput=False)
    ba2_d = dp("ba2", [P, DC_T], F32, isOutput=False)
    dwdg_d = dp("dwdg", [DC_T, P, N_PE_TAP * P], BF16, isOutput=False)
    dww_d = dp("dww", [P, DC_T, N_DVE_TAP], F32, isOutput=False)
    dwb_d = dp("dwb", [P, DC_T], F32, isOutput=False)
    gnw_d = dp("gnw", [P, DC_T], F32, isOutput=False)
    gnb_d = dp("gnb", [P, DC_T], F32, isOutput=False)
    pw2t_d = dp("pw2t", [DC, D], BF16, isOutput=False)
    pw2br_d = dp("pw2br", [1, D], BF16, isOutput=False)
    w1f2_d = dp("w1f2", [D, DF], BF16, isOutput=False)
    b1f2_d = dp("b1f2", [P, DF_T], F32, isOutput=False)
    w22_d = dp("w22", [DF, D], BF16, isOutput=False)
    b2r2_d = dp("b2r2", [1, D], BF16, isOutput=False)
    flnw_d = dp("flnw", [D], F32, isOutput=False)
    flnb_d = dp("flnb", [D], F32, isOutput=False)
    ident_d = dp("ident", [P, P], BF16, isOutput=False)
    zeros_d = dp("zeros", [128], BF16, isOutput=False)
    mask01_d = dp("mask01", [P, 12, 256], BF16, isOutput=False)
    out_d = dp("y_out", [T, D], F32, isOutput=True)

    with tile.TileContext(nc) as tc:
        with (
            tc.tile_pool(name="const", bufs=1) as cpool,
            tc.tile_pool(name="resid", bufs=1) as rpool,
            tc.tile_pool(name="lnt", bufs=2) as lpool,
            tc.tile_pool(name="big", bufs=1) as bigp,
            tc.tile_pool(name="w2res", bufs=1) as w2p,
            tc.tile_pool(name="wbig", bufs=2) as wbp,
            tc.tile_pool(name="med", bufs=1) as medp,
            tc.tile_pool(name="wd", bufs=2) as wdp,
            tc.tile_pool(name="small", bufs=4) as smp,
            tc.tile_pool(name="smx", bufs=6) as smxp,
            tc.tile_pool(name="psA", bufs=2, space="PSUM") as psA,
            tc.tile_pool(name="psB", bufs=2, space="PSUM") as psB,
            tc.tile_pool(name="psC", bufs=3, space="PSUM") as psC,
            tc.tile_pool(name="psS", bufs=1, space="PSUM") as psS,
        ):
            # ---------------- critical-path loads first ----------------
            ident = cpool.tile([P, P], BF16, tag="ident")
            nc.sync.dma_start(out=ident, in_=ident_d[:, :])
            identR = ident
            y = rpool.tile([P, TT_N, D], F32, tag="y")
            nc.sync.dma_start(out=y,
                              in_=x_d.rearrange("(a p) d -> p a d", p=P))
            eps_t = cpool.tile([P, 1], F32, tag="eps")
            nc.vector.memset(eps_t, EPS)

            def bcast_tile(dram_ap, tag):
                t_ = cpool.tile([P, D], F32, tag=tag)
                nc.sync.dma_start(out=t_, in_=_dram_vec_bcast_ap(dram_ap, D))
                return t_

            flnw_bc = bcast_tile(flnw_d[:], "flnw")
            flnb_bc = bcast_tile(flnb_d[:], "flnb")

            def brow(dram, tag):
                t_ = cpool.tile([1, D], BF16, tag=tag)
                nc.sync.dma_start(out=t_, in_=dram[:, :])
                return t_

            # bias rows, applied as K=1 rank-1 matmuls into the same PSUM
            b2r_t = brow(b2r_d, "b2r")
            outbr_t = brow(outbr_d, "outbr")
            pw2br_t = brow(pw2br_d, "pw2br")
            b2r2_t = brow(b2r2_d, "b2r2")
            ones_r = cpool.tile([1, P], BF16, tag="ones_r")
            nc.vector.memset(ones_r, 1.0)

            def c2d(dram, n, tag):
                t_ = cpool.tile([P, n], F32, tag=tag)
                nc.sync.dma_start(out=t_, in_=dram[:, :])
                return t_

            b1f_t = c2d(b1f_d, DF_T, "b1f")
            qb_t = c2d(qb_d, 4, "qb")
            kb_t = c2d(kb_d, 4, "kb")
            ba1_t = c2d(ba1_d, DC_T, "ba1")
            ba2_t = c2d(ba2_d, DC_T, "ba2")
            dwb_t = c2d(dwb_d, DC_T, "dwb")
            gnw_t = c2d(gnw_d, DC_T, "gnw")
            gnb_t = c2d(gnb_d, DC_T, "gnb")
            b1f2_t = c2d(b1f2_d, DF_T, "b1f2")
            dww_t = cpool.tile([P, DC_T, N_DVE_TAP], F32, tag="dww")
            nc.sync.dma_start(out=dww_t, in_=dww_d[:, :, :])

            ones_k = cpool.tile([P, 64], BF16, tag="ones_k")
            nc.vector.memset(ones_k, 1.0)

            zeros_ap = zeros_d[:]

            def zfill(out_ap, n1, n2):
                nc.sync.dma_start(out=out_ap, in_=bass.AP(
                    tensor=zeros_ap.tensor, offset=zeros_ap.offset,
                    ap=[[0, P], [0, n1], [1, n2]]))

            # ---------------- layernorm -> feature-major ----------------
            def ln_alloc(need_pad):
                """Fresh feature-major [128, D_T, KP_W] LN output tile (data
                goes at col PAD..PAD+T); pads zero-filled once at alloc."""
                ln_t = lpool.tile([P, D_T, KP_W], BF16, tag="lnt")
                if need_pad:
                    zfill(ln_t[:, :, 0:PAD], D_T, PAD)
                    zfill(ln_t[:, :, PAD + T:KP_W], D_T, KP_W - PAD - T)
                return ln_t

            def ln_tile(ln_t, tt):
                """LN of y[:, tt] (gamma/beta folded into next weights),
                PE-transposed into ln_t columns for time tile tt. Emitted
                inline right after y[:, tt] is produced so the PE pipeline
                never drains at module boundaries."""
                mv = smp.tile([P, 2], F32, tag="mv")
                st6 = smp.tile([P, 6], F32, tag="st6")
                nc.vector.bn_stats(out=st6, in_=y[:, tt, :])
                nc.vector.bn_aggr(out=mv, in_=st6)
                r_ = smp.tile([P, 1], F32, tag="r")
                nc.scalar.activation(out=r_, in_=mv[:, 1:2], func=AF.Sqrt,
                                     bias=eps_t, scale=1.0)
                nc.vector.reciprocal(out=r_, in_=r_)
                nmr = smp.tile([P, 1], F32, tag="nmr")
                nc.vector.tensor_scalar(out=nmr, in0=mv[:, 0:1], scalar1=r_,
                                        scalar2=-1.0, op0=OP.mult, op1=OP.mult)
                lnp = smp.tile([P, D], BF16, tag="lnp", bufs=2)
                nc.vector.tensor_scalar(out=lnp, in0=y[:, tt, :], scalar1=r_,
                                        scalar2=nmr, op0=OP.mult, op1=OP.add)
                tp = psB.tile([P, D], BF16, tag="tp")
                for dt in range(D_T):
                    nc.tensor.transpose(tp[:, dt * P:(dt + 1) * P],
                                        lnp[:, dt * P:(dt + 1) * P],
                                        identR)
                nc.scalar.activation(
                    out=ln_t[:, :, PAD + tt * P:PAD + (tt + 1) * P],
                    in_=tp.rearrange("p (a b) -> p a b", a=D_T),
                    func=AF.Copy)

            # ---------------- FFN (macaron half-residual) ----------------
            LAG = 2

            def ffn(w1_dram, b1_tile, w2_dram, b2row, ln_t, next_fn=None):
                w1t = wbp.tile([P, D_T, DF], BF16, tag="wbig")
                nc.scalar.dma_start(out=w1t,
                                  in_=w1_dram.rearrange("(a p) d -> p a d", p=P))
                w2r = None
                for th in range(2):
                    h1 = bigp.tile([P, DF_T, D], BF16, tag="big")
                    for ft in range(DF_T):
                        ps = psA.tile([P, D], F32, tag="mm")
                        for kt in range(D_T):
                            nc.tensor.matmul(
                                ps,
                                lhsT=w1t[:, kt, ft * P:(ft + 1) * P],
                                rhs=ln_t[:, kt, PAD + th * D:PAD + (th + 1) * D],
                                start=(kt == 0), stop=(kt == D_T - 1))
                        nc.scalar.activation(out=h1[:, ft, :], in_=ps, func=AF.Silu,
                                             bias=b1_tile[:, ft:ft + 1], scale=1.0)
                    if th == 0:
                        # w2 prefetch trigger sits behind the th=0 SiLUs on
                        # the ACT queue, staggering it past the critical
                        # x + w1 loads
                        w2r = w2p.tile([P, DF_T, D], BF16, tag="w2res")
                        nc.scalar.dma_start(
                            out=w2r,
                            in_=w2_dram.rearrange("(a p) d -> p a d", p=P))
                    for tc in range(4):
                        ps2 = psA.tile([P, D], F32, tag="mm")
                        for kt in range(DF_T):
                            nc.tensor.matmul(
                                ps2,
                                lhsT=h1[:, kt, tc * P:(tc + 1) * P],
                                rhs=w2r[:, kt, :],
                                start=(kt == 0), stop=False)
                        nc.tensor.matmul(ps2, lhsT=ones_r, rhs=b2row,
                                         start=False, stop=True)
                        g_tc = th * 4 + tc
                        nc.vector.scalar_tensor_tensor(
                            out=y[:, g_tc, :], in0=ps2, scalar=0.5,
                            in1=y[:, g_tc, :], op0=OP.mult, op1=OP.add)
                        if next_fn is not None and g_tc >= LAG:
                            next_fn(g_tc - LAG)
                if next_fn is not None:
                    for tt in range(TT_N - LAG, TT_N):
                        next_fn(tt)

            # ================= FFN1 =================
            ln1 = ln_alloc(False)
            for tt in range(TT_N):
                ln_tile(ln1, tt)
            attn_ln = ln_alloc(True)
            ffn(w1f_d, b1f_t, w2_d, b2r_t, ln1,
                next_fn=lambda tt: ln_tile(attn_ln, tt))

            # ================= attention =================
            ln_t = attn_ln
            qkvt = wbp.tile([P, D_T, 3 * D], BF16, tag="wbig")
            nc.scalar.dma_start(out=qkvt,
                              in_=qkvw_d.rearrange("(a p) d -> p a d", p=P))
            # per-qp k-major 0/1 masks, [128 k, 3 sb, 256 q] per qp
            mask01_t = cpool.tile([P, 12, 256], BF16, tag="mask01")
            nc.scalar.dma_start(out=mask01_t, in_=mask01_d[:, :, :])
            qk = bigp.tile([P, 4 * T + 4 * KP_W], BF16, tag="big")
            q_all = qk[:, 0:4 * T].rearrange("p (h t) -> p h t", h=4)
            kpad = qk[:, 4 * T:4 * T + 4 * KP_W].rearrange("p (h t) -> p h t", h=4)
            zfill(kpad[:, :, 0:PAD], 4, PAD)
            zfill(kpad[:, :, PAD + T:KP_W], 4, KP_W - PAD - T)
            # q, k: feature-major [head-pair 128, T]
            for hp in range(4):
                for tn in range(2):
                    psq = psA.tile([P, D], F32, tag="mm")
                    for kt in range(D_T):
                        nc.tensor.matmul(
                            psq,
                            lhsT=qkvt[:, kt, hp * P:(hp + 1) * P],
                            rhs=ln_t[:, kt, PAD + tn * D:PAD + (tn + 1) * D],
                            start=(kt == 0), stop=(kt == D_T - 1))
                    nc.scalar.activation(out=q_all[:, hp, tn * D:(tn + 1) * D],
                                         in_=psq, func=AF.Identity,
                                         bias=qb_t[:, hp:hp + 1], scale=1.0)
                    psk = psA.tile([P, D], F32, tag="mm")
                    for kt in range(D_T):
                        nc.tensor.matmul(
                            psk,
                            lhsT=qkvt[:, kt, D + hp * P:D + (hp + 1) * P],
                            rhs=ln_t[:, kt, PAD + tn * D:PAD + (tn + 1) * D],
                            start=(kt == 0), stop=(kt == D_T - 1))
                    nc.scalar.activation(
                        out=kpad[:, hp, PAD + tn * D:PAD + (tn + 1) * D],
                        in_=psk, func=AF.Identity,
                        bias=kb_t[:, hp:hp + 1], scale=1.0)
            # v: time-major, stored at +32 row offset (9 slots of 128)
            vpad = w2p.tile([P, 9, D], BF16, tag="w2res")
            for vt in range(9):
                psv = psA.tile([P, D], F32, tag="mm")
                for kt in range(D_T):
                    nc.tensor.matmul(
                        psv,
                        lhsT=ln_t[:, kt, vt * P:(vt + 1) * P],
                        rhs=qkvt[:, kt, 2 * D:3 * D],
                        start=(kt == 0), stop=(kt == D_T - 1))
                nc.scalar.activation(out=vpad[:, vt, :], in_=psv, func=AF.Copy)

            # k-major windowed attention: scores computed transposed [k, q] so
            # the AV matmul needs no probability transposes; softmax is
            # unnormalized exp, the 1/sum row scale is applied at the end via
            # a rank-1 broadcast matmul (sums come from a ones-vector matmul).
            o_t = [medp.tile([64, T], BF16, tag=f"med{i}", name=f"o_t{i}")
                   for i in range(8)]
            outwt = wbp.tile([64, 8, D], BF16, tag="wbig")
            nc.scalar.dma_start(out=outwt,
                              in_=outw_d.rearrange("(a p) d -> p a d", p=64))
            conv_ln = ln_alloc(False)
            # qp-major: after each 256-query block finishes for all 8 heads,
            # its two out-projection chains interleave into the PE stream so
            # the PE never starves on the softmax round-trips and the conv
            # module's LN starts 3 blocks early
            for qp in range(4):
                for hp in range(4):
                    for hi in range(2):
                        h = 2 * hp + hi
                        base = hi * 64
                        q_sl = q_all[base:base + 64, hp,
                                     qp * 256:(qp + 1) * 256]
                        # sb 0,1 batched in one PSUM bank; sb 2 separate
                        scp = psC.tile([P, 2, 256], F32, tag="sc")
                        for s in range(2):
                            vt = 2 * qp + s
                            nc.tensor.matmul(
                                scp[:, s, :],
                                lhsT=kpad[base:base + 64, hp, vt * P:(vt + 1) * P],
                                rhs=q_sl, start=True, stop=True)
                        sc2 = psC.tile([P, 2, 256], F32, tag="sc")
                        nc.tensor.matmul(
                            sc2[:, 0, :],
                            lhsT=kpad[base:base + 64, hp,
                                      (2 * qp + 2) * P:(2 * qp + 3) * P],
                            rhs=q_sl, start=True, stop=True)
                        exp2 = smxp.tile([P, 2, 256], BF16, tag="ex")
                        nc.scalar.activation(out=exp2, in_=scp, func=AF.Exp)
                        ex1 = smxp.tile([P, 256], BF16, tag="ex1")
                        nc.scalar.activation(out=ex1, in_=sc2[:, 0, :],
                                             func=AF.Exp)
                        smxb2 = smxp.tile([P, 2, 256], BF16, tag="smxb")
                        nc.vector.tensor_mul(
                            out=smxb2, in0=exp2,
                            in1=mask01_t[:, 3 * qp:3 * qp + 2, :])
                        smxb1 = smxp.tile([P, 256], BF16, tag="smxb1")
                        nc.vector.tensor_mul(out=smxb1, in0=ex1,
                                             in1=mask01_t[:, 3 * qp + 2, :])
                        smx_l = [smxb2[:, 0, :], smxb2[:, 1, :], smxb1]
                        # sums broadcast to all 64 partitions (M=64 ones)
                        sums = psS.tile([64, 256], F32, tag="sums")
                        for sb in range(3):
                            nc.tensor.matmul(sums, lhsT=ones_k, rhs=smx_l[sb],
                                             start=(sb == 0), stop=(sb == 2))
                        av = psB.tile([64, 256], F32, tag="tp", name=f"av{h}_{qp}")
                        for sb in range(3):
                            vt = 2 * qp + sb
                            nc.tensor.matmul(
                                av,
                                lhsT=vpad[:, vt, h * DH:(h + 1) * DH],
                                rhs=smx_l[sb],
                                start=(sb == 0), stop=(sb == 2))
                        r_ = smp.tile([64, 256], F32, tag="rrow")
                        nc.vector.reciprocal_approx_fast(out=r_, in_=sums)
                        nc.vector.tensor_mul(
                            out=o_t[h][:, qp * 256:(qp + 1) * 256],
                            in0=av, in1=r_)
                # out projection + residual for this query block
                for tc in (2 * qp, 2 * qp + 1):
                    pso = psA.tile([P, D], F32, tag="mm")
                    for h in range(8):
                        nc.tensor.matmul(
                            pso,
                            lhsT=o_t[h][:, tc * P:(tc + 1) * P],
                            rhs=outwt[:, h, :],
                            start=(h == 0), stop=False)
                    nc.tensor.matmul(pso, lhsT=ones_r, rhs=outbr_t,
                                     start=False, stop=True)
                    nc.vector.tensor_add(out=y[:, tc, :], in0=y[:, tc, :],
                                         in1=pso)
                    if tc >= LAG:
                        ln_tile(conv_ln, tc - LAG)
            for tt in range(TT_N - LAG, TT_N):
                ln_tile(conv_ln, tt)

            # ================= conv module =================
            ln_t = conv_ln
            pw1tt = wbp.tile([P, D_T, 2 * DC], BF16, tag="wbig")
            nc.scalar.dma_start(out=pw1tt,
                              in_=pw1t_d.rearrange("(a p) d -> p a d", p=P))
            g = bigp.tile([P, DC_T, G_W], BF16, tag="big")
            zfill(g[:, :, 0:CPAD], DC_T, CPAD)
            zfill(g[:, :, CPAD + T:G_W], DC_T, CPAD)
            for ct in range(DC_T):
                for tn in range(2):
                    ps_a2 = psA.tile([P, D], F32, tag="mm")
                    for kt in range(D_T):
                        nc.tensor.matmul(
                            ps_a2,
                            lhsT=pw1tt[:, kt, DC + ct * P:DC + (ct + 1) * P],
                            rhs=ln_t[:, kt, PAD + tn * D:PAD + (tn + 1) * D],
                            start=(kt == 0), stop=(kt == D_T - 1))
                    sig = smp.tile([P, D], F32, tag="sig", bufs=2)
                    nc.scalar.activation(out=sig, in_=ps_a2, func=AF.Sigmoid,
                                         bias=ba2_t[:, ct:ct + 1], scale=1.0)
                    ps_a1 = psA.tile([P, D], F32, tag="mm")
                    for kt in range(D_T):
                        nc.tensor.matmul(
                            ps_a1,
                            lhsT=pw1tt[:, kt, ct * P:(ct + 1) * P],
                            rhs=ln_t[:, kt, PAD + tn * D:PAD + (tn + 1) * D],
                            start=(kt == 0), stop=(kt == D_T - 1))
                    nc.vector.scalar_tensor_tensor(
                        out=g[:, ct, CPAD + tn * D:CPAD + (tn + 1) * D],
                        in0=ps_a1, scalar=ba1_t[:, ct:ct + 1], in1=sig,
                        op0=OP.add, op1=OP.mult)
            # depthwise conv split: N_PE_TAP taps as accumulating diagonal
            # matmuls on the PE, N_DVE_TAP taps as bf16 multiply-adds on the
            # DVE (full 1024-wide rows), joined by one STT per half
            cv = [medp.tile([P, T], BF16, tag=f"med{i}", name=f"cv{i}") for i in range(DC_T)]
            for ct in range(DC_T):
                dgt = wdp.tile([P, N_PE_TAP, P], BF16, tag="wd")
                nc.scalar.dma_start(out=dgt, in_=dwdg_d[ct, :, :].rearrange(
                    "p (j q) -> p j q", j=N_PE_TAP))
                # two parallel DVE accumulation chains to halve the latency
                acc = smp.tile([P, T], BF16, tag="dacc", bufs=2)
                acc2 = smp.tile([P, T], BF16, tag="dacc2", bufs=2)
                half = N_DVE_TAP // 2
                for idx, j in enumerate(DVE_TAPS):
                    gs = g[:, ct, j:j + T]
                    wj = dww_t[:, ct, idx:idx + 1]
                    dst = acc if idx < half else acc2
                    if idx == 0 or idx == half:
                        nc.vector.tensor_scalar(out=dst, in0=gs, scalar1=wj,
                                                scalar2=None, op0=OP.mult)
                    else:
                        nc.vector.scalar_tensor_tensor(
                            out=dst, in0=gs, scalar=wj, in1=dst,
                            op0=OP.mult, op1=OP.add)
                nc.vector.tensor_add(out=acc, in0=acc, in1=acc2)
                for tn in range(2):
                    ps = psA.tile([P, D], F32, tag="mm")
                    for jj in range(N_PE_TAP):
                        j = PE_TAPS[jj]
                        nc.tensor.matmul(
                            ps, lhsT=dgt[:, jj, :],
                            rhs=g[:, ct, j + tn * D:j + tn * D + D],
                            start=(jj == 0), stop=(jj == N_PE_TAP - 1))
                    nc.vector.scalar_tensor_tensor(
                        out=cv[ct][:, tn * D:(tn + 1) * D], in0=ps,
                        scalar=dwb_t[:, ct:ct + 1],
                        in1=acc[:, tn * D:(tn + 1) * D],
                        op0=OP.add, op1=OP.add)
            # GroupNorm(1 group over C,T) stats
            stats_pk = smp.tile([P, 16], F32, tag="stpk")
            for ct in range(DC_T):
                st = smp.tile([P, 2, 6], F32, tag="st26")
                nc.vector.bn_stats(out=st[:, 0, :], in_=cv[ct][:, 0:D])
                nc.vector.bn_stats(out=st[:, 1, :], in_=cv[ct][:, D:T])
                mv = smp.tile([P, 2], F32, tag="mv")
                nc.vector.bn_aggr(out=mv, in_=st)
                nc.vector.tensor_copy(out=stats_pk[:, ct:ct + 1], in_=mv[:, 0:1])
                nc.vector.scalar_tensor_tensor(
                    out=stats_pk[:, 8 + ct:9 + ct], in0=mv[:, 0:1],
                    scalar=mv[:, 0:1], in1=mv[:, 1:2], op0=OP.mult, op1=OP.add)
            red = smp.tile([P, 16], F32, tag="gred")
            nc.gpsimd.partition_all_reduce(red, stats_pk, channels=P,
                                           reduce_op=bass_isa.ReduceOp.add)
            sums = smp.tile([P, 2], F32, tag="sums")
            nc.vector.tensor_reduce(out=sums,
                                    in_=red.rearrange("p (a b) -> p a b", a=2),
                                    axis=AX.X, op=OP.add)
            mq = smp.tile([P, 2], F32, tag="mq")  # [mu, E[x^2]] on every partition
            nc.vector.tensor_scalar(out=mq, in0=sums, scalar1=1.0 / DC,
                                    scalar2=None, op0=OP.mult)
            # var = E[x^2] - mu^2
            var_t = smp.tile([P, 1], F32, tag="var")
            nc.vector.tensor_scalar(out=var_t, in0=mq[:, 0:1], scalar1=mq[:, 0:1],
                                    scalar2=-1.0, op0=OP.mult, op1=OP.mult)
            nc.vector.tensor_add(out=var_t, in0=var_t, in1=mq[:, 1:2])
            rstd = smp.tile([P, 1], F32, tag="rstd")
            nc.scalar.activation(out=rstd, in_=var_t, func=AF.Sqrt,
                                 bias=eps_t, scale=1.0)
            nc.vector.reciprocal(out=rstd, in_=rstd)
            # per-channel-tile scale/shift + SiLU, then pw2 + residual
            pw2tt = w2p.tile([P, DC_T, D], BF16, tag="w2res")
            nc.scalar.dma_start(out=pw2tt,
                              in_=pw2t_d.rearrange("(a p) d -> p a d", p=P))
            for ct in range(DC_T):
                s_c = smp.tile([P, 1], F32, tag="s_c")
                nc.vector.tensor_scalar(out=s_c, in0=gnw_t[:, ct:ct + 1],
                                        scalar1=rstd, scalar2=None,
                                        op0=OP.mult)
                t_c = smp.tile([P, 1], F32, tag="t_c")
                nc.vector.tensor_scalar(out=t_c, in0=s_c, scalar1=mq[:, 0:1],
                                        scalar2=-1.0, op0=OP.mult, op1=OP.mult)
                nc.vector.tensor_add(out=t_c, in0=t_c, in1=gnb_t[:, ct:ct + 1])
                nc.scalar.activation(out=cv[ct], in_=cv[ct], func=AF.Silu,
                                     bias=t_c, scale=s_c)
            ffn2_ln = ln_alloc(False)
            for tc in range(TT_N):
                psp = psA.tile([P, D], F32, tag="mm")
                for kt in range(DC_T):
                    nc.tensor.matmul(
                        psp,
                        lhsT=cv[kt][:, tc * P:(tc + 1) * P],
                        rhs=pw2tt[:, kt, :],
                        start=(kt == 0), stop=False)
                nc.tensor.matmul(psp, lhsT=ones_r, rhs=pw2br_t,
                                 start=False, stop=True)
                nc.vector.tensor_add(out=y[:, tc, :], in0=y[:, tc, :], in1=psp)
                if tc >= LAG:
                    ln_tile(ffn2_ln, tc - LAG)
            for tt in range(TT_N - LAG, TT_N):
                ln_tile(ffn2_ln, tt)

            # ================= FFN2 (final LN + store streamed per tile) ====
            def final_tile(tt):
                mv = smp.tile([P, 2], F32, tag="mv")
                st6 = smp.tile([P, 6], F32, tag="st6")
                nc.vector.bn_stats(out=st6, in_=y[:, tt, :])
                nc.vector.bn_aggr(out=mv, in_=st6)
                r_ = smp.tile([P, 1], F32, tag="r")
                nc.scalar.activation(out=r_, in_=mv[:, 1:2], func=AF.Sqrt,
                                     bias=eps_t, scale=1.0)
                nc.vector.reciprocal(out=r_, in_=r_)
                nmr = smp.tile([P, 1], F32, tag="nmr")
                nc.vector.tensor_scalar(out=nmr, in0=mv[:, 0:1], scalar1=r_,
                                        scalar2=-1.0, op0=OP.mult, op1=OP.mult)
                lnp = smp.tile([P, D], F32, tag="lnp", bufs=2)
                nc.vector.tensor_scalar(out=lnp, in0=y[:, tt, :], scalar1=r_,
                                        scalar2=nmr, op0=OP.mult, op1=OP.add)
                nc.vector.tensor_mul(out=lnp, in0=lnp, in1=flnw_bc)
                nc.vector.tensor_add(out=lnp, in0=lnp, in1=flnb_bc)
                nc.sync.dma_start(out=out_d[tt * P:(tt + 1) * P, :], in_=lnp)

            ffn(w1f2_d, b1f2_t, w22_d, b2r2_t, ffn2_ln, next_fn=final_tile)
    return nc


_NC_CACHE = {}


def _get_nc():
    if "nc" not in _NC_CACHE:
        nc = bacc.Bacc()
        _build(nc)
        nc.finalize()
        _NC_CACHE["nc"] = nc
    return _NC_CACHE["nc"]


def _prep_weights(inp):
    f = np.float32

    def a(x):
        return np.ascontiguousarray(np.asarray(x, dtype=f))

    def b(x):
        return np.ascontiguousarray(np.asarray(x, dtype=f).astype(NP_BF16))

    out = {}
    # FFN1: fold ln gamma/beta into w1/b1
    w1 = a(inp["ffn1_w1"]); lw = a(inp["ffn1_ln_w"]); lb = a(inp["ffn1_ln_b"])
    out["w1f"] = b(w1 * lw[:, None])
    b1 = a(inp["ffn1_b1"]) + lb @ w1
    out["b1f"] = a(b1.reshape(DF_T, P).T)
    out["w2"] = b(inp["ffn1_w2"])
    out["b2r"] = b(inp["ffn1_b2"])[None, :]
    # attention
    qkvw = a(inp["qkv_w"]); alw = a(inp["attn_ln_w"]); alb = a(inp["attn_ln_b"])
    qkvf = qkvw * alw[:, None]
    qkvb = a(inp["qkv_b"]) + alb @ qkvw
    scale = np.float32(DH ** -0.5)
    qkvf[:, :D] *= scale
    out["qkvw"] = b(qkvf)
    out["qb"] = a((qkvb[:D] * scale).reshape(4, P).T)
    out["kb"] = a(qkvb[D:2 * D].reshape(4, P).T)
    out["outw"] = b(inp["out_w"])
    # v-bias folded through the out projection (softmax weights sum to 1)
    out["outbr"] = b(a(inp["out_b"]) + qkvb[2 * D:] @ a(inp["out_w"]))[None, :]
    # conv module
    pw1 = a(inp["pw1_w"]); clw = a(inp["conv_ln_w"]); clb = a(inp["conv_ln_b"])
    out["pw1t"] = b((pw1 * clw[None, :]).T)
    pb = a(inp["pw1_b"]) + pw1 @ clb
    out["ba1"] = a(pb[:DC].reshape(DC_T, P).T)
    out["ba2"] = a(pb[DC:].reshape(DC_T, P).T)
    dw = a(inp["dw_w"]).reshape(DC, KTAP)
    dg = np.zeros((DC_T, N_PE_TAP, P, P), dtype=f)
    idx = np.arange(P)
    for ct in range(DC_T):
        for jj, j in enumerate(PE_TAPS):
            dg[ct, jj, idx, idx] = dw[ct * P:(ct + 1) * P, j]
    out["dwdg"] = b(dg.transpose(0, 2, 1, 3).reshape(DC_T, P, N_PE_TAP * P))
    out["dww"] = a(dw.reshape(DC_T, P, KTAP).transpose(1, 0, 2)[:, :, DVE_TAPS])
    out["dwb"] = a(a(inp["dw_b"]).reshape(DC_T, P).T)
    out["gnw"] = a(a(inp["gn_w"]).reshape(DC_T, P).T)
    out["gnb"] = a(a(inp["gn_b"]).reshape(DC_T, P).T)
    out["pw2t"] = b(a(inp["pw2_w"]).T)
    out["pw2br"] = b(inp["pw2_b"])[None, :]
    # FFN2
    w12 = a(inp["ffn2_w1"]); lw2 = a(inp["ffn2_ln_w"]); lb2 = a(inp["ffn2_ln_b"])
    out["w1f2"] = b(w12 * lw2[:, None])
    b12 = a(inp["ffn2_b1"]) + lb2 @ w12
    out["b1f2"] = a(b12.reshape(DF_T, P).T)
    out["w22"] = b(inp["ffn2_w2"])
    out["b2r2"] = b(inp["ffn2_b2"])[None, :]
    out["flnw"] = a(inp["final_ln_w"])
    out["flnb"] = a(inp["final_ln_b"])
    out["ident"] = np.eye(P, dtype=f).astype(NP_BF16)
    out["zeros"] = np.zeros(128, dtype=NP_BF16)
    # per-qp k-major multiplicative masks: [12 = 4 qp x 3 sb, 128 k, 256 q]
    # key time for (sb, k) = sb*128 + k - PAD; valid iff |key - q| <= W/2
    kk = np.arange(P)[:, None]
    nn = np.arange(256)[None, :]
    w2_ = WIN // 2
    m = np.zeros((12, P, 256), dtype=f)
    for qp in range(4):
        for sb in range(3):
            msk = (np.abs(sb * P + kk - PAD - nn) <= w2_).astype(f)
            if qp == 0 and sb == 0:
                msk *= (kk >= PAD)      # key >= 0
            if qp == 3 and sb == 2:
                msk *= (kk < PAD)       # key < T
            m[qp * 3 + sb] = msk
    out["mask01"] = b(m.transpose(1, 0, 2))
    return out


def kernel(**inputs):
    x = np.asarray(inputs["x"], dtype=np.float32)
    assert x.shape == (B, T, D)
    weights = _prep_weights(inputs)
    nc = _get_nc()
    in_maps = []
    for i in range(N_CORES):
        m = dict(weights)
        m["x"] = np.ascontiguousarray(x[i])
        in_maps.append(m)
    res = run_bass_kernel_spmd(nc, in_maps, core_ids=list(range(N_CORES)))
    outs = [res.results[i]["y_out"] for i in range(N_CORES)]
    return np.stack(outs, axis=0).astype(np.float32)


if __name__ == "__main__":
    rng = np.random.default_rng(0)
    pass



# revision 64
# speedup vs baseline: 1.0175x; 1.0175x over previous
"""ConformerBlock Trainium2 kernel.

Data-parallel over batch: B=8 = one batch element per NeuronCore, no
collectives (every module in the block is per-sample, including the
GroupNorm which normalizes over (C,T) of each sample).

Per-core layout strategy:
  - residual `y` kept time-major [T=1024, D=512] as [128, 8, 512] SBUF tile
  - LayerNorm stats via bn_stats per 128-row time tile; gamma/beta folded
    into the *following* matmul's weights on the host. The LN + transpose
    for the NEXT module is emitted inline (lagged 2 tiles) inside each
    module's residual-update loop so the in-order PE queue never drains at
    module boundaries (keeps the HAM clock gate at 8/8).
  - normalized tiles are PE-transposed to feature-major [D, T]; weights
    stationary bf16, activations moving bf16, N=512 per matmul. Output
    biases are applied as K=1 rank-1 (ones-row) matmuls into the same
    PSUM accumulation, keeping bias adds off the critical DVE/Pool path.
  - windowed attention (|i-j|<=32) computed k-major: scores [k, q] via
    3 kpad-tile matmuls per 256-query block (sb pair batched in one PSUM
    bank), exp on ScalarE, multiplicative 0/1 masks on DVE (bf16 2x),
    denominators via an M=64 ones matmul (broadcast across partitions),
    reciprocal_approx_fast, unnormalized AV, then one [64,256] multiply.
    No probability transposes and no PSUM->SBUF probability copies.
  - depthwise conv K=31: 27 taps as accumulating diagonal matmuls on the
    PE (weights streamed from DRAM), 4 even-offset taps as two parallel
    bf16 multiply-add chains on the DVE, joined at the PSUM drain.
  - weight prefetches ride the scalar-engine DMA queue so they overlap
    the x/residual load on the sync queue.
"""

import numpy as np
import ml_dtypes

import concourse.bass as bass
import concourse.bacc as bacc
import concourse.tile as tile
from concourse import mybir
from concourse.bass_utils import run_bass_kernel_spmd
from concourse import bass_isa

F32 = mybir.dt.float32
F32R = mybir.dt.float32r
BF16 = mybir.dt.bfloat16
NP_BF16 = ml_dtypes.bfloat16
AF = mybir.ActivationFunctionType
OP = mybir.AluOpType
AX = mybir.AxisListType

B, T, D, H, KTAP, WIN = 8, 1024, 512, 8, 31, 64
DF = 4 * D            # 2048 ffn hidden
DC = 2 * D            # 1024 conv channels
DH = D // H           # 64
EPS = 1e-5
P = 128
TT_N = T // P         # 8 time tiles
D_T = D // P          # 4
DF_T = DF // P        # 16
DC_T = DC // P        # 8
PAD = 32              # ln_t / kpad leading pad
KP_W = PAD + T + 96   # 1152 padded time width (feature-major)
CPAD = 15             # conv halo
G_W = T + 2 * CPAD    # 1054
NEG = -30000.0

N_CORES = 8

# depthwise taps: a few even offsets run on the DVE (two parallel
# accumulation chains), the rest accumulate on the PE as diagonal matmuls
DVE_TAPS = [4, 8, 12, 16]
PE_TAPS = [j for j in range(KTAP) if j not in DVE_TAPS]
N_PE_TAP = len(PE_TAPS)
N_DVE_TAP = len(DVE_TAPS)


def _dram_vec_bcast_ap(dram_ap, n):
    """AP reading a [n] dram vector broadcast across 128 partitions."""
    return bass.AP(tensor=dram_ap.tensor, offset=dram_ap.offset,
                   ap=[[0, P], [1, n]])


def _build(nc):
    dp = nc.declare_dram_parameter
    x_d = dp("x", [T, D], F32, isOutput=False)
    w1f_d = dp("w1f", [D, DF], BF16, isOutput=False)
    b1f_d = dp("b1f", [P, DF_T], F32, isOutput=False)
    w2_d = dp("w2", [DF, D], BF16, isOutput=False)
    b2r_d = dp("b2r", [1, D], BF16, isOutput=False)
    qkvw_d = dp("qkvw", [D, 3 * D], BF16, isOutput=False)
    qb_d = dp("qb", [P, 4], F32, isOutput=False)
    kb_d = dp("kb", [P, 4], F32, isOutput=False)
    outw_d = dp("outw", [D, D], BF16, isOutput=False)
    outbr_d = dp("outbr", [1, D], BF16, isOutput=False)
    pw1t_d = dp("pw1t", [D, 2 * DC], BF16, isOutput=False)
    ba1_d = dp("ba1", [P, DC_T], F32, isOutput=False)
    ba2_d = dp("ba2", [P, DC_T], F32, isOutput=False)
    dwdg_d = dp("dwdg", [DC_T, P, N_PE_TAP * P], BF16, isOutput=False)
    dww_d = dp("dww", [P, DC_T, N_DVE_TAP], F32, isOutput=False)
    dwb_d = dp("dwb", [P, DC_T], F32, isOutput=False)
    gnw_d = dp("gnw", [P, DC_T], F32, isOutput=False)
    gnb_d = dp("gnb", [P, DC_T], F32, isOutput=False)
    pw2t_d = dp("pw2t", [DC, D], BF16, isOutput=False)
    pw2br_d = dp("pw2br", [1, D], BF16, isOutput=False)
    w1f2_d = dp("w1f2", [D, DF], BF16, isOutput=False)
    b1f2_d = dp("b1f2", [P, DF_T], F32, isOutput=False)
    w22_d = dp("w22", [DF, D], BF16, isOutput=False)
    b2r2_d = dp("b2r2", [1, D], BF16, isOutput=False)
    flnw_d = dp("flnw", [D], F32, isOutput=False)
    flnb_d = dp("flnb", [D], F32, isOutput=False)
    ident_d = dp("ident", [P, P], BF16, isOutput=False)
    zeros_d = dp("zeros", [128], BF16, isOutput=False)
    mask01_d = dp("mask01", [P, 12, 256], BF16, isOutput=False)
    out_d = dp("y_out", [T, D], F32, isOutput=True)

    with tile.TileContext(nc) as tc:
        with (
            tc.tile_pool(name="const", bufs=1) as cpool,
            tc.tile_pool(name="resid", bufs=1) as rpool,
            tc.tile_pool(name="lnt", bufs=2) as lpool,
            tc.tile_pool(name="big", bufs=2) as bigp,
            tc.tile_pool(name="w2res", bufs=1) as w2p,
            tc.tile_pool(name="wbig", bufs=2) as wbp,
            tc.tile_pool(name="med", bufs=1) as medp,
            tc.tile_pool(name="wd", bufs=2) as wdp,
            tc.tile_pool(name="small", bufs=4) as smp,
            tc.tile_pool(name="smx", bufs=6) as smxp,
            tc.tile_pool(name="psA", bufs=2, space="PSUM") as psA,
            tc.tile_pool(name="psB", bufs=2, space="PSUM") as psB,
            tc.tile_pool(name="psC", bufs=3, space="PSUM") as psC,
            tc.tile_pool(name="psS", bufs=1, space="PSUM") as psS,
        ):
            # ---------------- critical-path loads first ----------------
            ident = cpool.tile([P, P], BF16, tag="ident")
            nc.sync.dma_start(out=ident, in_=ident_d[:, :])
            identR = ident
            y = rpool.tile([P, TT_N, D], F32, tag="y")
            nc.sync.dma_start(out=y,
                              in_=x_d.rearrange("(a p) d -> p a d", p=P))
            eps_t = cpool.tile([P, 1], F32, tag="eps")
            nc.vector.memset(eps_t, EPS)

            def bcast_tile(dram_ap, tag):
                t_ = cpool.tile([P, D], F32, tag=tag)
                nc.sync.dma_start(out=t_, in_=_dram_vec_bcast_ap(dram_ap, D))
                return t_

            flnw_bc = bcast_tile(flnw_d[:], "flnw")
            flnb_bc = bcast_tile(flnb_d[:], "flnb")

            def brow(dram, tag):
                t_ = cpool.tile([1, D], BF16, tag=tag)
                nc.sync.dma_start(out=t_, in_=dram[:, :])
                return t_

            # bias rows, applied as K=1 rank-1 matmuls into the same PSUM
            b2r_t = brow(b2r_d, "b2r")
            outbr_t = brow(outbr_d, "outbr")
            pw2br_t = brow(pw2br_d, "pw2br")
            b2r2_t = brow(b2r2_d, "b2r2")
            ones_r = cpool.tile([1, P], BF16, tag="ones_r")
            nc.vector.memset(ones_r, 1.0)

            def c2d(dram, n, tag):
                t_ = cpool.tile([P, n], F32, tag=tag)
                nc.sync.dma_start(out=t_, in_=dram[:, :])
                return t_

            b1f_t = c2d(b1f_d, DF_T, "b1f")
            qb_t = c2d(qb_d, 4, "qb")
            kb_t = c2d(kb_d, 4, "kb")
            ba1_t = c2d(ba1_d, DC_T, "ba1")
            ba2_t = c2d(ba2_d, DC_T, "ba2")
            dwb_t = c2d(dwb_d, DC_T, "dwb")
            gnw_t = c2d(gnw_d, DC_T, "gnw")
            gnb_t = c2d(gnb_d, DC_T, "gnb")
            b1f2_t = c2d(b1f2_d, DF_T, "b1f2")
            dww_t = cpool.tile([P, DC_T, N_DVE_TAP], F32, tag="dww")
            nc.sync.dma_start(out=dww_t, in_=dww_d[:, :, :])

            ones_k = cpool.tile([P, 64], BF16, tag="ones_k")
            nc.vector.memset(ones_k, 1.0)

            zeros_ap = zeros_d[:]

            def zfill(out_ap, n1, n2):
                nc.sync.dma_start(out=out_ap, in_=bass.AP(
                    tensor=zeros_ap.tensor, offset=zeros_ap.offset,
                    ap=[[0, P], [0, n1], [1, n2]]))

            # ---------------- layernorm -> feature-major ----------------
            def ln_alloc(need_pad):
                """Fresh feature-major [128, D_T, KP_W] LN output tile (data
                goes at col PAD..PAD+T); pads zero-filled once at alloc."""
                ln_t = lpool.tile([P, D_T, KP_W], BF16, tag="lnt")
                if need_pad:
                    zfill(ln_t[:, :, 0:PAD], D_T, PAD)
                    zfill(ln_t[:, :, PAD + T:KP_W], D_T, KP_W - PAD - T)
                return ln_t

            def ln_tile(ln_t, tt):
                """LN of y[:, tt] (gamma/beta folded into next weights),
                PE-transposed into ln_t columns for time tile tt. Emitted
                inline right after y[:, tt] is produced so the PE pipeline
                never drains at module boundaries."""
                mv = smp.tile([P, 2], F32, tag="mv")
                st6 = smp.tile([P, 6], F32, tag="st6")
                nc.vector.bn_stats(out=st6, in_=y[:, tt, :])
                nc.vector.bn_aggr(out=mv, in_=st6)
                r_ = smp.tile([P, 1], F32, tag="r")
                nc.scalar.activation(out=r_, in_=mv[:, 1:2], func=AF.Sqrt,
                                     bias=eps_t, scale=1.0)
                nc.vector.reciprocal(out=r_, in_=r_)
                nmr = smp.tile([P, 1], F32, tag="nmr")
                nc.vector.tensor_scalar(out=nmr, in0=mv[:, 0:1], scalar1=r_,
                                        scalar2=-1.0, op0=OP.mult, op1=OP.mult)
                lnp = smp.tile([P, D], BF16, tag="lnp", bufs=2)
                nc.vector.tensor_scalar(out=lnp, in0=y[:, tt, :], scalar1=r_,
                                        scalar2=nmr, op0=OP.mult, op1=OP.add)
                tp = psB.tile([P, D], BF16, tag="tp")
                for dt in range(D_T):
                    nc.tensor.transpose(tp[:, dt * P:(dt + 1) * P],
                                        lnp[:, dt * P:(dt + 1) * P],
                                        identR)
                nc.scalar.activation(
                    out=ln_t[:, :, PAD + tt * P:PAD + (tt + 1) * P],
                    in_=tp.rearrange("p (a b) -> p a b", a=D_T),
                    func=AF.Copy)

            # ---------------- FFN (macaron half-residual) ----------------
            LAG = 2

            def ffn(w1_dram, b1_tile, w2_dram, b2row, ln_t, next_fn=None):
                w1t = wbp.tile([P, D_T, DF], BF16, tag="wbig")
                nc.scalar.dma_start(out=w1t,
                                  in_=w1_dram.rearrange("(a p) d -> p a d", p=P))
                w2r = None
                for th in range(2):
                    h1 = bigp.tile([P, DF_T, D], BF16, tag="big")
                    for ft in range(DF_T):
                        ps = psA.tile([P, D], F32, tag="mm")
                        for kt in range(D_T):
                            nc.tensor.matmul(
                                ps,
                                lhsT=w1t[:, kt, ft * P:(ft + 1) * P],
                                rhs=ln_t[:, kt, PAD + th * D:PAD + (th + 1) * D],
                                start=(kt == 0), stop=(kt == D_T - 1))
                        nc.scalar.activation(out=h1[:, ft, :], in_=ps, func=AF.Silu,
                                             bias=b1_tile[:, ft:ft + 1], scale=1.0)
                    if th == 0:
                        # w2 prefetch trigger sits behind the th=0 SiLUs on
                        # the ACT queue, staggering it past the critical
                        # x + w1 loads
                        w2r = w2p.tile([P, DF_T, D], BF16, tag="w2res")
                        nc.scalar.dma_start(
                            out=w2r,
                            in_=w2_dram.rearrange("(a p) d -> p a d", p=P))
                    for tc in range(4):
                        ps2 = psA.tile([P, D], F32, tag="mm")
                        for kt in range(DF_T):
                            nc.tensor.matmul(
                                ps2,
                                lhsT=h1[:, kt, tc * P:(tc + 1) * P],
                                rhs=w2r[:, kt, :],
                                start=(kt == 0), stop=False)
                        nc.tensor.matmul(ps2, lhsT=ones_r, rhs=b2row,
                                         start=False, stop=True)
                        g_tc = th * 4 + tc
                        nc.vector.scalar_tensor_tensor(
                            out=y[:, g_tc, :], in0=ps2, scalar=0.5,
                            in1=y[:, g_tc, :], op0=OP.mult, op1=OP.add)
                        if next_fn is not None and g_tc >= LAG:
                            next_fn(g_tc - LAG)
                if next_fn is not None:
                    for tt in range(TT_N - LAG, TT_N):
                        next_fn(tt)

            # ================= FFN1 =================
            ln1 = ln_alloc(False)
            for tt in range(TT_N):
                ln_tile(ln1, tt)
            attn_ln = ln_alloc(True)
            ffn(w1f_d, b1f_t, w2_d, b2r_t, ln1,
                next_fn=lambda tt: ln_tile(attn_ln, tt))

            # ================= attention =================
            ln_t = attn_ln
            qkvt = wbp.tile([P, D_T, 3 * D], BF16, tag="wbig")
            nc.scalar.dma_start(out=qkvt,
                              in_=qkvw_d.rearrange("(a p) d -> p a d", p=P))
            # per-qp k-major 0/1 masks, [128 k, 3 sb, 256 q] per qp
            mask01_t = cpool.tile([P, 12, 256], BF16, tag="mask01")
            nc.scalar.dma_start(out=mask01_t, in_=mask01_d[:, :, :])
            qk = bigp.tile([P, 4 * T + 4 * KP_W], BF16, tag="big")
            q_all = qk[:, 0:4 * T].rearrange("p (h t) -> p h t", h=4)
            kpad = qk[:, 4 * T:4 * T + 4 * KP_W].rearrange("p (h t) -> p h t", h=4)
            zfill(kpad[:, :, 0:PAD], 4, PAD)
            zfill(kpad[:, :, PAD + T:KP_W], 4, KP_W - PAD - T)
            # q, k: feature-major [head-pair 128, T]
            for hp in range(4):
                for tn in range(2):
                    psq = psA.tile([P, D], F32, tag="mm")
                    for kt in range(D_T):
                        nc.tensor.matmul(
                            psq,
                            lhsT=qkvt[:, kt, hp * P:(hp + 1) * P],
                            rhs=ln_t[:, kt, PAD + tn * D:PAD + (tn + 1) * D],
                            start=(kt == 0), stop=(kt == D_T - 1))
                    nc.scalar.activation(out=q_all[:, hp, tn * D:(tn + 1) * D],
                                         in_=psq, func=AF.Identity,
                                         bias=qb_t[:, hp:hp + 1], scale=1.0)
                    psk = psA.tile([P, D], F32, tag="mm")
                    for kt in range(D_T):
                        nc.tensor.matmul(
                            psk,
                            lhsT=qkvt[:, kt, D + hp * P:D + (hp + 1) * P],
                            rhs=ln_t[:, kt, PAD + tn * D:PAD + (tn + 1) * D],
                            start=(kt == 0), stop=(kt == D_T - 1))
                    nc.scalar.activation(
                        out=kpad[:, hp, PAD + tn * D:PAD + (tn + 1) * D],
                        in_=psk, func=AF.Identity,
                        bias=kb_t[:, hp:hp + 1], scale=1.0)
            # v: time-major, stored at +32 row offset (9 slots of 128)
            vpad = w2p.tile([P, 9, D], BF16, tag="w2res")
            for vt in range(9):
                psv = psA.tile([P, D], F32, tag="mm")
                for kt in range(D_T):
                    nc.tensor.matmul(
                        psv,
                        lhsT=ln_t[:, kt, vt * P:(vt + 1) * P],
                        rhs=qkvt[:, kt, 2 * D:3 * D],
                        start=(kt == 0), stop=(kt == D_T - 1))
                nc.scalar.activation(out=vpad[:, vt, :], in_=psv, func=AF.Copy)

            # k-major windowed attention: scores computed transposed [k, q] so
            # the AV matmul needs no probability transposes; softmax is
            # unnormalized exp, the 1/sum row scale is applied at the end via
            # a rank-1 broadcast matmul (sums come from a ones-vector matmul).
            o_t = [medp.tile([64, T], BF16, tag=f"med{i}", name=f"o_t{i}")
                   for i in range(8)]
            outwt = wbp.tile([64, 8, D], BF16, tag="wbig")
            nc.scalar.dma_start(out=outwt,
                              in_=outw_d.rearrange("(a p) d -> p a d", p=64))
            conv_ln = ln_alloc(False)
            # qp-major: after each 256-query block finishes for all 8 heads,
            # its two out-projection chains interleave into the PE stream so
            # the PE never starves on the softmax round-trips and the conv
            # module's LN starts 3 blocks early
            for qp in range(4):
                for hp in range(4):
                    for hi in range(2):
                        h = 2 * hp + hi
                        base = hi * 64
                        q_sl = q_all[base:base + 64, hp,
                                     qp * 256:(qp + 1) * 256]
                        # sb 0,1 batched in one PSUM bank; sb 2 separate
                        scp = psC.tile([P, 2, 256], F32, tag="sc")
                        for s in range(2):
                            vt = 2 * qp + s
                            nc.tensor.matmul(
                                scp[:, s, :],
                                lhsT=kpad[base:base + 64, hp, vt * P:(vt + 1) * P],
                                rhs=q_sl, start=True, stop=True)
                        sc2 = psC.tile([P, 2, 256], F32, tag="sc")
                        nc.tensor.matmul(
                            sc2[:, 0, :],
                            lhsT=kpad[base:base + 64, hp,
                                      (2 * qp + 2) * P:(2 * qp + 3) * P],
                            rhs=q_sl, start=True, stop=True)
                        exp2 = smxp.tile([P, 2, 256], BF16, tag="ex")
                        nc.scalar.activation(out=exp2, in_=scp, func=AF.Exp)
                        ex1 = smxp.tile([P, 256], BF16, tag="ex1")
                        nc.scalar.activation(out=ex1, in_=sc2[:, 0, :],
                                             func=AF.Exp)
                        smxb2 = smxp.tile([P, 2, 256], BF16, tag="smxb")
                        nc.vector.tensor_mul(
                            out=smxb2, in0=exp2,
                            in1=mask01_t[:, 3 * qp:3 * qp + 2, :])
                        smxb1 = smxp.tile([P, 256], BF16, tag="smxb1")
                        nc.vector.tensor_mul(out=smxb1, in0=ex1,
                                             in1=mask01_t[:, 3 * qp + 2, :])
                        smx_l = [smxb2[:, 0, :], smxb2[:, 1, :], smxb1]
                        # sums broadcast to all 64 partitions (M=64 ones)
                        sums = psS.tile([64, 256], F32, tag="sums")
                        for sb in range(3):
                            nc.tensor.matmul(sums, lhsT=ones_k, rhs=smx_l[sb],
                                             start=(sb == 0), stop=(sb == 2))
                        av = psB.tile([64, 256], F32, tag="tp", name=f"av{h}_{qp}")
                        for sb in range(3):
                            vt = 2 * qp + sb
                            nc.tensor.matmul(
                                av,
                                lhsT=vpad[:, vt, h * DH:(h + 1) * DH],
                                rhs=smx_l[sb],
                                start=(sb == 0), stop=(sb == 2))
                        r_ = smp.tile([64, 256], F32, tag="rrow")
                        nc.vector.reciprocal_approx_fast(out=r_, in_=sums)
                        nc.vector.tensor_mul(
                            out=o_t[h][:, qp * 256:(qp + 1) * 256],
                            in0=av, in1=r_)
                # out projection + residual for this query block
                for tc in (2 * qp, 2 * qp + 1):
                    pso = psA.tile([P, D], F32, tag="mm")
                    for h in range(8):
                        nc.tensor.matmul(
                            pso,
                            lhsT=o_t[h][:, tc * P:(tc + 1) * P],
                            rhs=outwt[:, h, :],
                            start=(h == 0), stop=False)
                    nc.tensor.matmul(pso, lhsT=ones_r, rhs=outbr_t,
                                     start=False, stop=True)
                    nc.vector.tensor_add(out=y[:, tc, :], in0=y[:, tc, :],
                                         in1=pso)
                    if tc >= LAG:
                        ln_tile(conv_ln, tc - LAG)
            for tt in range(TT_N - LAG, TT_N):
                ln_tile(conv_ln, tt)

            # ================= conv module =================
            ln_t = conv_ln
            pw1tt = wbp.tile([P, D_T, 2 * DC], BF16, tag="wbig")
            nc.scalar.dma_start(out=pw1tt,
                              in_=pw1t_d.rearrange("(a p) d -> p a d", p=P))
            g = bigp.tile([P, DC_T, G_W], BF16, tag="big")
            zfill(g[:, :, 0:CPAD], DC_T, CPAD)
            zfill(g[:, :, CPAD + T:G_W], DC_T, CPAD)
            for ct in range(DC_T):
                for tn in range(2):
                    ps_a2 = psA.tile([P, D], F32, tag="mm")
                    for kt in range(D_T):
                        nc.tensor.matmul(
                            ps_a2,
                            lhsT=pw1tt[:, kt, DC + ct * P:DC + (ct + 1) * P],
                            rhs=ln_t[:, kt, PAD + tn * D:PAD + (tn + 1) * D],
                            start=(kt == 0), stop=(kt == D_T - 1))
                    sig = smp.tile([P, D], F32, tag="sig", bufs=2)
                    nc.scalar.activation(out=sig, in_=ps_a2, func=AF.Sigmoid,
                                         bias=ba2_t[:, ct:ct + 1], scale=1.0)
                    ps_a1 = psA.tile([P, D], F32, tag="mm")
                    for kt in range(D_T):
                        nc.tensor.matmul(
                            ps_a1,
                            lhsT=pw1tt[:, kt, ct * P:(ct + 1) * P],
                            rhs=ln_t[:, kt, PAD + tn * D:PAD + (tn + 1) * D],
                            start=(kt == 0), stop=(kt == D_T - 1))
                    nc.vector.scalar_tensor_tensor(
                        out=g[:, ct, CPAD + tn * D:CPAD + (tn + 1) * D],
                        in0=ps_a1, scalar=ba1_t[:, ct:ct + 1], in1=sig,
                        op0=OP.add, op1=OP.mult)
            # depthwise conv split: N_PE_TAP taps as accumulating diagonal
            # matmuls on the PE, N_DVE_TAP taps as bf16 multiply-adds on the
            # DVE (full 1024-wide rows), joined by one STT per half
            cv = [medp.tile([P, T], BF16, tag=f"med{i}", name=f"cv{i}") for i in range(DC_T)]
            for ct in range(DC_T):
                dgt = wdp.tile([P, N_PE_TAP, P], BF16, tag="wd")
                nc.scalar.dma_start(out=dgt, in_=dwdg_d[ct, :, :].rearrange(
                    "p (j q) -> p j q", j=N_PE_TAP))
                # two parallel DVE accumulation chains to halve the latency
                acc = smp.tile([P, T], BF16, tag="dacc", bufs=2)
                acc2 = smp.tile([P, T], BF16, tag="dacc2", bufs=2)
                half = N_DVE_TAP // 2
                for idx, j in enumerate(DVE_TAPS):
                    gs = g[:, ct, j:j + T]
                    wj = dww_t[:, ct, idx:idx + 1]
                    dst = acc if idx < half else acc2
                    if idx == 0 or idx == half:
                        nc.vector.tensor_scalar(out=dst, in0=gs, scalar1=wj,
                                                scalar2=None, op0=OP.mult)
                    else:
                        nc.vector.scalar_tensor_tensor(
                            out=dst, in0=gs, scalar=wj, in1=dst,
                            op0=OP.mult, op1=OP.add)
                nc.vector.tensor_add(out=acc, in0=acc, in1=acc2)
                for tn in range(2):
                    ps = psA.tile([P, D], F32, tag="mm")
                    for jj in range(N_PE_TAP):
                        j = PE_TAPS[jj]
                        nc.tensor.matmul(
                            ps, lhsT=dgt[:, jj, :],
                            rhs=g[:, ct, j + tn * D:j + tn * D + D],
                            start=(jj == 0), stop=(jj == N_PE_TAP - 1))
                    nc.vector.scalar_tensor_tensor(
                        out=cv[ct][:, tn * D:(tn + 1) * D], in0=ps,
                        scalar=dwb_t[:, ct:ct + 1],
                        in1=acc[:, tn * D:(tn + 1) * D],
                        op0=OP.add, op1=OP.add)
            # GroupNorm(1 group over C,T) stats
            stats_pk = smp.tile([P, 16], F32, tag="stpk")
            for ct in range(DC_T):
                st = smp.tile([P, 2, 6], F32, tag="st26")
                nc.vector.bn_stats(out=st[:, 0, :], in_=cv[ct][:, 0:D])
                nc.vector.bn_stats(out=st[:, 1, :], in_=cv[ct][:, D:T])
                mv = smp.tile([P, 2], F32, tag="mv")
                nc.vector.bn_aggr(out=mv, in_=st)
                nc.vector.tensor_copy(out=stats_pk[:, ct:ct + 1], in_=mv[:, 0:1])
                nc.vector.scalar_tensor_tensor(
                    out=stats_pk[:, 8 + ct:9 + ct], in0=mv[:, 0:1],
                    scalar=mv[:, 0:1], in1=mv[:, 1:2], op0=OP.mult, op1=OP.add)
            red = smp.tile([P, 16], F32, tag="gred")
            nc.gpsimd.partition_all_reduce(red, stats_pk, channels=P,
                                           reduce_op=bass_isa.ReduceOp.add)
            sums = smp.tile([P, 2], F32, tag="sums")
            nc.vector.tensor_reduce(out=sums,
                                    in_=red.rearrange("p (a b) -> p a b", a=2),
                                    axis=AX.X, op=OP.add)
            mq = smp.tile([P, 2], F32, tag="mq")  # [mu, E[x^2]] on every partition
            nc.vector.tensor_scalar(out=mq, in0=sums, scalar1=1.0 / DC,
                                    scalar2=None, op0=OP.mult)
            # var = E[x^2] - mu^2
            var_t = smp.tile([P, 1], F32, tag="var")
            nc.vector.tensor_scalar(out=var_t, in0=mq[:, 0:1], scalar1=mq[:, 0:1],
                                    scalar2=-1.0, op0=OP.mult, op1=OP.mult)
            nc.vector.tensor_add(out=var_t, in0=var_t, in1=mq[:, 1:2])
            rstd = smp.tile([P, 1], F32, tag="rstd")
            nc.scalar.activation(out=rstd, in_=var_t, func=AF.Sqrt,
                                 bias=eps_t, scale=1.0)
            nc.vector.reciprocal(out=rstd, in_=rstd)
            # per-channel-tile scale/shift + SiLU, then pw2 + residual
            pw2tt = w2p.tile([P, DC_T, D], BF16, tag="w2res")
            nc.scalar.dma_start(out=pw2tt,
                              in_=pw2t_d.rearrange("(a p) d -> p a d", p=P))
            for ct in range(DC_T):
                s_c = smp.tile([P, 1], F32, tag="s_c")
                nc.vector.tensor_scalar(out=s_c, in0=gnw_t[:, ct:ct + 1],
                                        scalar1=rstd, scalar2=None,
                                        op0=OP.mult)
                t_c = smp.tile([P, 1], F32, tag="t_c")
                nc.vector.tensor_scalar(out=t_c, in0=s_c, scalar1=mq[:, 0:1],
                                        scalar2=-1.0, op0=OP.mult, op1=OP.mult)
                nc.vector.tensor_add(out=t_c, in0=t_c, in1=gnb_t[:, ct:ct + 1])
                nc.scalar.activation(out=cv[ct], in_=cv[ct], func=AF.Silu,
                                     bias=t_c, scale=s_c)
            ffn2_ln = ln_alloc(False)
            for tc in range(TT_N):
                psp = psA.tile([P, D], F32, tag="mm")
                for kt in range(DC_T):
                    nc.tensor.matmul(
                        psp,
                        lhsT=cv[kt][:, tc * P:(tc + 1) * P],
                        rhs=pw2tt[:, kt, :],
                        start=(kt == 0), stop=False)
                nc.tensor.matmul(psp, lhsT=ones_r, rhs=pw2br_t,
                                 start=False, stop=True)
                nc.vector.tensor_add(out=y[:, tc, :], in0=y[:, tc, :], in1=psp)
                if tc >= LAG:
                    ln_tile(ffn2_ln, tc - LAG)
            for tt in range(TT_N - LAG, TT_N):
                ln_tile(ffn2_ln, tt)

            # ================= FFN2 (final LN + store streamed per tile) ====
            def final_tile(tt):
                mv = smp.tile([P, 2], F32, tag="mv")
                st6 = smp.tile([P, 6], F32, tag="st6")
                nc.vector.bn_stats(out=st6, in_=y[:, tt, :])
                nc.vector.bn_aggr(out=mv, in_=st6)
                r_ = smp.tile([P, 1], F32, tag="r")
                nc.scalar.activation(out=r_, in_=mv[:, 1:2], func=AF.Sqrt,
                                     bias=eps_t, scale=1.0)
                nc.vector.reciprocal(out=r_, in_=r_)
                nmr = smp.tile([P, 1], F32, tag="nmr")
                nc.vector.tensor_scalar(out=nmr, in0=mv[:, 0:1], scalar1=r_,
                                        scalar2=-1.0, op0=OP.mult, op1=OP.mult)
                lnp = smp.tile([P, D], F32, tag="lnp", bufs=2)
                nc.vector.tensor_scalar(out=lnp, in0=y[:, tt, :], scalar1=r_,
                                        scalar2=nmr, op0=OP.mult, op1=OP.add)
                nc.vector.tensor_mul(out=lnp, in0=lnp, in1=flnw_bc)
                nc.vector.tensor_add(out=lnp, in0=lnp, in1=flnb_bc)
                nc.sync.dma_start(out=out_d[tt * P:(tt + 1) * P, :], in_=lnp)

            ffn(w1f2_d, b1f2_t, w22_d, b2r2_t, ffn2_ln, next_fn=final_tile)
    return nc


_NC_CACHE = {}


def _get_nc():
    if "nc" not in _NC_CACHE:
        nc = bacc.Bacc()
        _build(nc)
        nc.finalize()
        _NC_CACHE["nc"] = nc
    return _NC_CACHE["nc"]


def _prep_weights(inp):
    f = np.float32

    def a(x):
        return np.ascontiguousarray(np.asarray(x, dtype=f))

    def b(x):
        return np.ascontiguousarray(np.asarray(x, dtype=f).astype(NP_BF16))

    out = {}
    # FFN1: fold ln gamma/beta into w1/b1
    w1 = a(inp["ffn1_w1"]); lw = a(inp["ffn1_ln_w"]); lb = a(inp["ffn1_ln_b"])
    out["w1f"] = b(w1 * lw[:, None])
    b1 = a(inp["ffn1_b1"]) + lb @ w1
    out["b1f"] = a(b1.reshape(DF_T, P).T)
    out["w2"] = b(inp["ffn1_w2"])
    out["b2r"] = b(inp["ffn1_b2"])[None, :]
    # attention
    qkvw = a(inp["qkv_w"]); alw = a(inp["attn_ln_w"]); alb = a(inp["attn_ln_b"])
    qkvf = qkvw * alw[:, None]
    qkvb = a(inp["qkv_b"]) + alb @ qkvw
    scale = np.float32(DH ** -0.5)
    qkvf[:, :D] *= scale
    out["qkvw"] = b(qkvf)
    out["qb"] = a((qkvb[:D] * scale).reshape(4, P).T)
    out["kb"] = a(qkvb[D:2 * D].reshape(4, P).T)
    out["outw"] = b(inp["out_w"])
    # v-bias folded through the out projection (softmax weights sum to 1)
    out["outbr"] = b(a(inp["out_b"]) + qkvb[2 * D:] @ a(inp["out_w"]))[None, :]
    # conv module
    pw1 = a(inp["pw1_w"]); clw = a(inp["conv_ln_w"]); clb = a(inp["conv_ln_b"])
    out["pw1t"] = b((pw1 * clw[None, :]).T)
    pb = a(inp["pw1_b"]) + pw1 @ clb
    out["ba1"] = a(pb[:DC].reshape(DC_T, P).T)
    out["ba2"] = a(pb[DC:].reshape(DC_T, P).T)
    dw = a(inp["dw_w"]).reshape(DC, KTAP)
    dg = np.zeros((DC_T, N_PE_TAP, P, P), dtype=f)
    idx = np.arange(P)
    for ct in range(DC_T):
        for jj, j in enumerate(PE_TAPS):
            dg[ct, jj, idx, idx] = dw[ct * P:(ct + 1) * P, j]
    out["dwdg"] = b(dg.transpose(0, 2, 1, 3).reshape(DC_T, P, N_PE_TAP * P))
    out["dww"] = a(dw.reshape(DC_T, P, KTAP).transpose(1, 0, 2)[:, :, DVE_TAPS])
    out["dwb"] = a(a(inp["dw_b"]).reshape(DC_T, P).T)
    out["gnw"] = a(a(inp["gn_w"]).reshape(DC_T, P).T)
    out["gnb"] = a(a(inp["gn_b"]).reshape(DC_T, P).T)
    out["pw2t"] = b(a(inp["pw2_w"]).T)
    out["pw2br"] = b(inp["pw2_b"])[None, :]
    # FFN2
    w12 = a(inp["ffn2_w1"]); lw2 = a(inp["ffn2_ln_w"]); lb2 = a(inp["ffn2_ln_b"])
    out["w1f2"] = b(w12 * lw2[:, None])
    b12 = a(inp["ffn2_b1"]) + lb2 @ w12
    out["b1f2"] = a(b12.reshape(DF_T, P).T)
    out["w22"] = b(inp["ffn2_w2"])
    out["b2r2"] = b(inp["ffn2_b2"])[None, :]
    out["flnw"] = a(inp["final_ln_w"])
    out["flnb"] = a(inp["final_ln_b"])
    out["ident"] = np.eye(P, dtype=f).astype(NP_BF16)
    out["zeros"] = np.zeros(128, dtype=NP_BF16)
    # per-qp k-major multiplicative masks: [12 = 4 qp x 3 sb, 128 k, 256 q]
    # key time for (sb, k) = sb*128 + k - PAD; valid iff |key - q| <= W/2
    kk = np.arange(P)[:, None]
    nn = np.arange(256)[None, :]
    w2_ = WIN // 2
    m = np.zeros((12, P, 256), dtype=f)
    for qp in range(4):
        for sb in range(3):
            msk = (np.abs(sb * P + kk - PAD - nn) <= w2_).astype(f)
            if qp == 0 and sb == 0:
                msk *= (kk >= PAD)      # key >= 0
            if qp == 3 and sb == 2:
                msk *= (kk < PAD)       # key < T
            m[qp * 3 + sb] = msk
    out["mask01"] = b(m.transpose(1, 0, 2))
    return out


def kernel(**inputs):
    x = np.asarray(inputs["x"], dtype=np.float32)
    assert x.shape == (B, T, D)
    weights = _prep_weights(inputs)
    nc = _get_nc()
    in_maps = []
    for i in range(N_CORES):
        m = dict(weights)
        m["x"] = np.ascontiguousarray(x[i])
        in_maps.append(m)
    res = run_bass_kernel_spmd(nc, in_maps, core_ids=list(range(N_CORES)))
    outs = [res.results[i]["y_out"] for i in range(N_CORES)]
    return np.stack(outs, axis=0).astype(np.float32)


if __name__ == "__main__":
    rng = np.random.default_rng(0)
    pass

